# revision 1
# baseline (speedup 1.0000x reference)
"""Trainium2 Bass kernel for the LoRA-BC block (nn_LoRABCBlock).

Computation (per reference):
    base = x @ w_base.T
    h = layernorm(x) * gamma + beta
    qkv = h @ w_qkv.T ; attention (2 heads, head_dim 32) over full sequence
    attn_out = attn_output @ w_attn_out.T
    delta = ((h + attn_out) @ lora_down) @ lora_up
    out = base + (1/8) * delta

Sharding: data-parallel over (batch, seq-half) -> 8 cores. Each core owns
1024 query rows of one batch element, computes k/v over that batch's full
2048-row sequence (the other half's rows are re-normalized locally - cheap),
and produces its 1024 output rows. Weights are replicated. No collectives.

All matmuls run in bf16 on the TensorEngine with fp32 PSUM accumulation.
Activations are produced transposed (feature-on-partition) via TensorE
transposes so every matmul contracts over the partition dimension.
"""

import sys

sys.path.insert(0, "/opt/trn_rl_repo")

from contextlib import ExitStack

import numpy as np

import concourse.bass as bass
import concourse.tile as tile
from concourse import bacc, mybir
from concourse.bass_utils import run_bass_kernel_spmd
from concourse.masks import make_identity

F32 = mybir.dt.float32
BF16 = mybir.dt.bfloat16
AF = mybir.ActivationFunctionType

E = 1024          # embed dim
DM = 1024         # d_model
R = 8             # lora rank
SCALING = 1.0 / R
DA = 64           # attn dim
NH = 2            # heads
HD = DA // NH     # head dim = 32
SOWN = 1024       # rows owned per core
SFULL = 2048      # rows per batch element
NC = 8            # cores
P = 128
KT = E // P       # 8 k-tiles
MT = SOWN // P    # 8 own m-tiles
ST = SFULL // P   # 16 sequence tiles
ATT_SCALE = float(HD) ** -0.5


def build_kernel():
    nc = bacc.Bacc("TRN2", target_bir_lowering=False, debug=False, num_devices=NC)

    x_own = nc.dram_tensor("x_own", [SOWN, E], F32, kind="ExternalInput").ap()
    x_oth = nc.dram_tensor("x_oth", [SOWN, E], F32, kind="ExternalInput").ap()
    w_base = nc.dram_tensor("w_base", [DM, E], F32, kind="ExternalInput").ap()
    ln_g = nc.dram_tensor("ln_g", [E], F32, kind="ExternalInput").ap()
    ln_b = nc.dram_tensor("ln_b", [E], F32, kind="ExternalInput").ap()
    ld = nc.dram_tensor("ld", [E, R], F32, kind="ExternalInput").ap()
    lu = nc.dram_tensor("lu", [R, DM], F32, kind="ExternalInput").ap()
    w_qkv = nc.dram_tensor("w_qkv", [3 * DA, E], F32, kind="ExternalInput").ap()
    w_ao = nc.dram_tensor("w_ao", [E, DA], F32, kind="ExternalInput").ap()
    out_d = nc.dram_tensor("out", [SOWN, DM], F32, kind="ExternalOutput").ap()

    with tile.TileContext(nc) as tc, ExitStack() as ctx:
        persist = ctx.enter_context(tc.tile_pool(name="persist", bufs=1))
        ld_pool = ctx.enter_context(tc.tile_pool(name="loads", bufs=3))
        zh_pool = ctx.enter_context(tc.tile_pool(name="zh", bufs=2))
        xh_pool = ctx.enter_context(tc.tile_pool(name="xh", bufs=2))
        st_pool = ctx.enter_context(tc.tile_pool(name="stats", bufs=4))
        p_pool = ctx.enter_context(tc.tile_pool(name="probs", bufs=2))
        pt_pool = ctx.enter_context(tc.tile_pool(name="probsT", bufs=2))
        ao_pool = ctx.enter_context(tc.tile_pool(name="ao", bufs=2))
        o_pool = ctx.enter_context(tc.tile_pool(name="outs", bufs=2))
        ps = ctx.enter_context(tc.tile_pool(name="ps", bufs=1, space="PSUM"))

        _psn = [0]

        def ps_tile(shape, dtype, tag, bufs=None):
            _psn[0] += 1
            if bufs is None:
                bufs = {"tp": 2, "mm": 2, "scores": 1}.get(tag, 1)
            return ps.tile(shape, dtype, tag=tag, bufs=bufs,
                           name=f"ps_{tag}_{_psn[0]}")

        # ---------------- phase 0: constants + weights ----------------
        ident = persist.tile([P, P], BF16, tag="ident")
        make_identity(nc, ident)

        eps_t = persist.tile([P, 1], F32, tag="eps")
        nc.vector.memset(eps_t, 1e-5)

        # gamma/beta arranged [p, kt] (k = kt*128 + p)
        gT = persist.tile([P, KT], F32, tag="gT")
        bT = persist.tile([P, KT], F32, tag="bT")
        nc.sync.dma_start(out=gT, in_=ln_g.rearrange("(kt p) -> p kt", p=P))
        nc.sync.dma_start(out=bT, in_=ln_b.rearrange("(kt p) -> p kt", p=P))

        # w_base -> WbT[kt] [128k, 1024n] bf16
        WbT = [persist.tile([P, DM], BF16, tag=f"WbT{k}", name=f"WbT{k}") for k in range(KT)]
        for ntile in range(KT):
            wf = ld_pool.tile([P, E], F32, tag="wload")
            nc.sync.dma_start(out=wf, in_=w_base[ntile * P:(ntile + 1) * P, :])
            wh = ld_pool.tile([P, E], BF16, tag="wcast")
            nc.vector.tensor_copy(out=wh, in_=wf)
            for k in range(KT):
                tp = ps_tile([P, P], BF16, "tp")
                nc.tensor.transpose(tp, wh[:, k * P:(k + 1) * P], ident)
                nc.vector.tensor_copy(out=WbT[k][:, ntile * P:(ntile + 1) * P], in_=tp)

        # w_qkv -> wqkvT[kt] [128k, 192a] bf16
        wqkvT = [persist.tile([P, 3 * DA], BF16, tag=f"wqkvT{k}", name=f"wqkvT{k}") for k in range(KT)]
        wq0f = ld_pool.tile([P, E], F32, tag="wload")
        nc.sync.dma_start(out=wq0f, in_=w_qkv[0:P, :])
        wq0h = persist.tile([P, E], BF16, tag="wq0h")
        nc.vector.tensor_copy(out=wq0h, in_=wq0f)
        wq1f = ld_pool.tile([64, E], F32, tag="wload1")
        nc.sync.dma_start(out=wq1f, in_=w_qkv[P:3 * DA, :])
        wq1h = persist.tile([64, E], BF16, tag="wq1h")
        nc.vector.tensor_copy(out=wq1h, in_=wq1f)
        for k in range(KT):
            tp = ps_tile([P, P], BF16, "tp")
            nc.tensor.transpose(tp, wq0h[:, k * P:(k + 1) * P], ident)
            nc.vector.tensor_copy(out=wqkvT[k][:, 0:P], in_=tp)
            tp2 = ps_tile([P, 64], BF16, "tp")
            nc.tensor.transpose(tp2, wq1h[:, k * P:(k + 1) * P], ident[0:64, 0:64])
            nc.vector.tensor_copy(out=wqkvT[k][:, P:3 * DA], in_=tp2)

        # w_attn_out -> waoT [64d, 1024n] bf16
        waoT = persist.tile([DA, E], BF16, tag="waoT")
        for ntile in range(KT):
            wf = ld_pool.tile([P, DA], F32, tag="waoload")
            nc.sync.dma_start(out=wf, in_=w_ao[ntile * P:(ntile + 1) * P, :])
            wh = ld_pool.tile([P, DA], BF16, tag="waocast")
            nc.vector.tensor_copy(out=wh, in_=wf)
            tp = ps_tile([DA, P], BF16, "tp")
            nc.tensor.transpose(tp, wh, ident)
            nc.vector.tensor_copy(out=waoT[:, ntile * P:(ntile + 1) * P], in_=tp)

        # lora_down [E, R] -> [p, kt, r] bf16 (k on partitions, natural lhsT)
        ld_f = ld_pool.tile([P, KT, R], F32, tag="ldload")
        nc.sync.dma_start(out=ld_f, in_=ld.rearrange("(kt p) r -> p kt r", p=P))
        ld_sb = persist.tile([P, KT, R], BF16, tag="ld_sb")
        nc.vector.tensor_copy(out=ld_sb, in_=ld_f)

        # lora_up [R, DM] bf16, pre-scaled by SCALING
        lu_f = ld_pool.tile([R, DM], F32, tag="luload")
        nc.sync.dma_start(out=lu_f, in_=lu)
        lu_sb = persist.tile([R, DM], BF16, tag="lu_sb")
        nc.scalar.mul(lu_sb, lu_f, SCALING)

        # ---------------- persistent activations ----------------
        hT = [persist.tile([P, SFULL], BF16, tag=f"hT{k}", name=f"hT{k}") for k in range(KT)]
        xT = [persist.tile([P, SOWN], BF16, tag=f"xT{k}", name=f"xT{k}") for k in range(KT)]
        qT = persist.tile([DA, SOWN], BF16, tag="qT")
        kTt = persist.tile([DA, SFULL], BF16, tag="kTt")
        v_sb = persist.tile([P, ST, DA], BF16, tag="v_sb")
        aoT = persist.tile([DA, SOWN], BF16, tag="aoT")
        hwaT = [persist.tile([P, SOWN], BF16, tag=f"hwaT{k}", name=f"hwaT{k}") for k in range(KT)]
        tT = persist.tile([R, SOWN], BF16, tag="tT")

        # ---------------- phase 1: layernorm + transposes ----------------
        for st in range(ST):
            own = st < MT
            src = x_own if own else x_oth
            row0 = st * P if own else (st - MT) * P
            xf = ld_pool.tile([P, E], F32, tag="xin")
            nc.sync.dma_start(out=xf, in_=src[row0:row0 + P, :])

            stats = st_pool.tile([P, 2, 6], F32, tag="bnstats")
            xr = xf.rearrange("p (n f) -> p n f", f=512)
            for sg in range(2):
                nc.vector.bn_stats(out=stats[:, sg, :], in_=xr[:, sg, :])
            mv = st_pool.tile([P, 2], F32, tag="mv")
            nc.vector.bn_aggr(out=mv, in_=stats)

            # rstd = 1/sqrt(var+eps); nmr = -mu*rstd
            rstd = st_pool.tile([P, 1], F32, tag="rstd")
            nc.scalar.activation(out=rstd, in_=mv[:, 1:2], func=AF.Sqrt, bias=eps_t)
            nc.vector.reciprocal(out=rstd, in_=rstd)
            nmr = st_pool.tile([P, 1], F32, tag="nmr")
            nc.vector.tensor_scalar(out=nmr, in0=mv[:, 0:1], scalar1=rstd,
                                    scalar2=-1.0, op0=mybir.AluOpType.mult,
                                    op1=mybir.AluOpType.mult)
            # z = (x - mu) * rstd   (bf16)
            zh = zh_pool.tile([P, E], BF16, tag="zh")
            nc.scalar.activation(out=zh, in_=xf, func=AF.Identity,
                                 scale=rstd, bias=nmr)
            if own:
                xh = xh_pool.tile([P, E], BF16, tag="xh")
                nc.scalar.copy(out=xh, in_=xf)

            for k in range(KT):
                tp = ps_tile([P, P], BF16, "tp")
                nc.tensor.transpose(tp, zh[:, k * P:(k + 1) * P], ident)
                # h = z*gamma + beta  (gamma/beta per-partition in T layout)
                nc.scalar.activation(out=hT[k][:, st * P:(st + 1) * P], in_=tp,
                                     func=AF.Identity, scale=gT[:, k:k + 1],
                                     bias=bT[:, k:k + 1])
                if own:
                    tp2 = ps_tile([P, P], BF16, "tp")
                    nc.tensor.transpose(tp2, xh[:, k * P:(k + 1) * P], ident)
                    nc.vector.tensor_copy(out=xT[k][:, st * P:(st + 1) * P], in_=tp2)

        # ---------------- phase 2: qkv projections ----------------
        # q,k for own rows: psum [128a, 512m], accumulate over kt
        for grp in range(SOWN // 512):
            pq = ps_tile([P, 512], F32, "mm")
            for k in range(KT):
                nc.tensor.matmul(pq, wqkvT[k][:, 0:P],
                                 hT[k][:, grp * 512:(grp + 1) * 512],
                                 start=(k == 0), stop=(k == KT - 1))
            nc.vector.tensor_copy(out=qT[:, grp * 512:(grp + 1) * 512], in_=pq[0:DA, :])
            nc.vector.tensor_copy(out=kTt[:, grp * 512:(grp + 1) * 512], in_=pq[DA:P, :])
        # k for other rows
        for grp in range(SOWN // 512):
            pk = ps_tile([DA, 512], F32, "mm")
            for k in range(KT):
                nc.tensor.matmul(pk, wqkvT[k][:, DA:P],
                                 hT[k][:, SOWN + grp * 512:SOWN + (grp + 1) * 512],
                                 start=(k == 0), stop=(k == KT - 1))
            nc.vector.tensor_copy(out=kTt[:, SOWN + grp * 512:SOWN + (grp + 1) * 512],
                                  in_=pk)
        # v natural [j, d] for all 16 j-tiles
        for jt in range(ST):
            pv = ps_tile([P, DA], F32, "mm")
            for k in range(KT):
                nc.tensor.matmul(pv, hT[k][:, jt * P:(jt + 1) * P],
                                 wqkvT[k][:, P:3 * DA],
                                 start=(k == 0), stop=(k == KT - 1))
            nc.vector.tensor_copy(out=v_sb[:, jt, :], in_=pv)

        # ---------------- phase 3: attention ----------------
        for mt in range(MT):
            ao_t = ao_pool.tile([P, DA], BF16, tag="ao_t")
            for h in range(NH):
                d0 = h * HD
                ps_s = ps_tile([P, SFULL], F32, "scores")
                for nsk in range(SFULL // 512):
                    nc.tensor.matmul(ps_s[:, nsk * 512:(nsk + 1) * 512],
                                     qT[d0:d0 + HD, mt * P:(mt + 1) * P],
                                     kTt[d0:d0 + HD, nsk * 512:(nsk + 1) * 512],
                                     start=True, stop=True)
                p_t = p_pool.tile([P, SFULL], BF16, tag="p_t")
                rs = st_pool.tile([P, 1], F32, tag="rs")
                nc.scalar.activation(out=p_t, in_=ps_s, func=AF.Exp,
                                     scale=ATT_SCALE, accum_out=rs)
                rr = st_pool.tile([P, 1], F32, tag="rr")
                nc.vector.reciprocal(out=rr, in_=rs)
                pT_t = pt_pool.tile([P, ST, P], BF16, tag="pT_t")
                for jt in range(ST):
                    tp = ps_tile([P, P], BF16, "tp")
                    nc.tensor.transpose(tp, p_t[:, jt * P:(jt + 1) * P], ident)
                    nc.vector.tensor_copy(out=pT_t[:, jt, :], in_=tp)
                pao = ps_tile([P, DA], F32, "mm")
                for jt in range(ST):
                    nc.tensor.matmul(pao[:, d0:d0 + HD], pT_t[:, jt, :],
                                     v_sb[:, jt, d0:d0 + HD],
                                     start=(jt == 0), stop=(jt == ST - 1))
                nc.vector.tensor_scalar_mul(out=ao_t[:, d0:d0 + HD],
                                            in0=pao[:, d0:d0 + HD], scalar1=rr)
            tp = ps_tile([DA, P], BF16, "tp")
            nc.tensor.transpose(tp, ao_t, ident)
            nc.vector.tensor_copy(out=aoT[:, mt * P:(mt + 1) * P], in_=tp)

        # ---------------- phase 4: attn_out projection + residual ----------------
        for ntile in range(KT):
            for grp in range(SOWN // 512):
                p4 = ps_tile([P, 512], F32, "mm")
                nc.tensor.matmul(p4, waoT[:, ntile * P:(ntile + 1) * P],
                                 aoT[:, grp * 512:(grp + 1) * 512],
                                 start=True, stop=True)
                nc.vector.tensor_add(out=hwaT[ntile][:, grp * 512:(grp + 1) * 512],
                                     in0=p4,
                                     in1=hT[ntile][:, grp * 512:(grp + 1) * 512])

        # ---------------- phase 5: lora down ----------------
        for mt in range(MT):
            p5 = ps_tile([R, P], F32, "mm")
            for k in range(KT):
                nc.tensor.matmul(p5, ld_sb[:, k, :], hwaT[k][:, mt * P:(mt + 1) * P],
                                 start=(k == 0), stop=(k == KT - 1))
            nc.vector.tensor_copy(out=tT[:, mt * P:(mt + 1) * P], in_=p5)

        # ---------------- phase 6: base matmul + lora up + output ----------------
        for mt in range(MT):
            o_t = o_pool.tile([P, DM], F32, tag="o_t")
            for grp in range(DM // 512):
                p6 = ps_tile([P, 512], F32, "mm")
                for k in range(KT):
                    nc.tensor.matmul(p6, xT[k][:, mt * P:(mt + 1) * P],
                                     WbT[k][:, grp * 512:(grp + 1) * 512],
                                     start=(k == 0), stop=False)
                nc.tensor.matmul(p6, tT[:, mt * P:(mt + 1) * P],
                                 lu_sb[:, grp * 512:(grp + 1) * 512],
                                 start=False, stop=True)
                nc.scalar.activation(out=o_t[:, grp * 512:(grp + 1) * 512],
                                     in_=p6, func=AF.Copy)
            nc.sync.dma_start(out=out_d[mt * P:(mt + 1) * P, :], in_=o_t)

    nc.compile()
    return nc


_NC_CACHE = None


def _get_nc():
    global _NC_CACHE
    if _NC_CACHE is None:
        _NC_CACHE = build_kernel()
    return _NC_CACHE


def kernel(x, w_base, ln_gamma, ln_beta, lora_down, lora_up, w_qkv, w_attn_out,
           _trace=False):
    x = np.ascontiguousarray(np.asarray(x, dtype=np.float32))
    wk = {
        "w_base": np.ascontiguousarray(np.asarray(w_base, np.float32)),
        "ln_g": np.ascontiguousarray(np.asarray(ln_gamma, np.float32)),
        "ln_b": np.ascontiguousarray(np.asarray(ln_beta, np.float32)),
        "ld": np.ascontiguousarray(np.asarray(lora_down, np.float32)),
        "lu": np.ascontiguousarray(np.asarray(lora_up, np.float32)),
        "w_qkv": np.ascontiguousarray(np.asarray(w_qkv, np.float32)),
        "w_ao": np.ascontiguousarray(np.asarray(w_attn_out, np.float32)),
    }
    nc = _get_nc()
    in_maps = []
    for c in range(NC):
        b, half = divmod(c, 2)
        own = np.ascontiguousarray(x[b, half * SOWN:(half + 1) * SOWN])
        oth = np.ascontiguousarray(x[b, (1 - half) * SOWN:(2 - half) * SOWN])
        in_maps.append({"x_own": own, "x_oth": oth, **wk})
    res = run_bass_kernel_spmd(nc, in_maps, core_ids=list(range(NC)), trace=_trace)
    B, S = x.shape[0], x.shape[1]
    out = np.empty((B, S, DM), np.float32)
    for c in range(NC):
        b, half = divmod(c, 2)
        out[b, half * SOWN:(half + 1) * SOWN] = res.results[c]["out"]
    if _trace:
        kernel.last_exec_time_ns = res.exec_time_ns
        kernel.last_results = res
    return out

